# revision 102
# baseline (speedup 1.0000x reference)
"""Trainium2 Bass kernel for AttentionWithComplexRoPE.

Strategy (8 NeuronCores): data-parallel over batch (B=2) x tensor-parallel
over heads (16 heads -> 4 per core). Core c handles batch c//4, heads
[4*(c%4), 4*(c%4)+4).

Per-core pipeline (bf16 x/Wq/Wk/Wv + projections, f32r elsewhere):
  phase 1: q/k projections (bf16, K=128 tiles) + RoPE on DVE, then DMA
           repack into per-chunk head-pair tensors qri/kri [h r|i] so QK
           contracts K=64 in a single matmul per head; v (+ones col) into
           parity-split tensors (even heads f32r, odd heads bf16), evicted
           by ScalarE. DMA issue order matches PE consumption order.
  phase 2: per 512-q chunk, TWO passes (head pair 01, then 23). Each pass
           streams 16 k-tiles; the two heads' scores live in two
           INDEPENDENT 1-bank double-buffered PSUM tiles: ScalarE computes
           exact exp for the even head while DVE computes a bf16
           Schraudolph bit-trick exp for the odd head — two decoupled
           producer/consumer chains that never gate each other. PV is
           emitted two k-tiles behind exp so the in-order PE queue never
           blocks. Ones-column denominator; each pass's normalize is
           deferred and spread as four small thunks over the FOLLOWING
           pass's k-tiles so the DVE FIFO never sees a burst.
  phase 3: Wo with K=128 head-pair stationaries; each chunk's 8 matmul
           halves ride the NEXT chunk's pass-0 (the last chunk's ride its
           own pass 1), borrowing whichever PV accumulator banks are idle;
           ScalarE/DVE evict; DMA out.
Host: slice/permute weights, transpose x, replicate freqs; sum the 4
per-batch partials at the end.

Modeled per-core exec time 194.5us (baseline 432.4us); max rel err 1.18e-2.
"""
import sys

if "/opt/trn_rl_repo" not in sys.path:
    sys.path.insert(0, "/opt/trn_rl_repo")

import ml_dtypes
import numpy as np

import concourse.bass as bass
import concourse.mybir as mybir
import concourse.tile as tile
from concourse import bacc
from concourse.bass_utils import run_bass_kernel_spmd

F32 = mybir.dt.float32
F32R = mybir.dt.float32r
BF16 = mybir.dt.bfloat16
NP_BF16 = ml_dtypes.bfloat16

B, S, C = 2, 2048, 1024
H = 16                      # global heads
HL = 4                      # heads per core
D = C // H                  # 64
DH = 32                     # complex pairs per head
F = HL * D                  # 256 local features
N_CORES = 8
KT = C // 128               # 8 contraction tiles for projections
TT = S // 128               # 16 token tiles
SC = S // 512               # 4 s-chunks
SCALE = float(D) ** -0.5
WO_SLOTS = (8, 9, 10, 11, 12, 13, 14, 15)   # pass-0 k-tiles carrying Wo halves
# Odd heads compute exp on DVE via the bf16 Schraudolph bit-trick
# exp(z) ~= bitcast_bf16(uint16(2^7/ln2 * z + (127*2^7 - 7.42)));
# softmax normalization absorbs most of the ~2% element error
SCHRAUD_A = SCALE * (2.0 ** 7 / float(np.log(2.0)))
SCHRAUD_B = float(127 * 2 ** 7) - 486411.0 / 2 ** 16

_CACHED_NC = None


def build_module():
    nc = bacc.Bacc("TRN2", target_bir_lowering=False)

    xt = nc.dram_tensor("xt", [C, S], BF16, kind="ExternalInput")
    wqr = nc.dram_tensor("wqr", [128, KT * 128], BF16, kind="ExternalInput")
    wqi = nc.dram_tensor("wqi", [128, KT * 128], BF16, kind="ExternalInput")
    wkr = nc.dram_tensor("wkr", [128, KT * 128], BF16, kind="ExternalInput")
    wki = nc.dram_tensor("wki", [128, KT * 128], BF16, kind="ExternalInput")
    wv = nc.dram_tensor("wv", [128, KT * 256], BF16, kind="ExternalInput")
    wo = nc.dram_tensor("wo", [2, 128, C], F32R, kind="ExternalInput")
    fr = nc.dram_tensor("fr", [128, S], F32, kind="ExternalInput")
    fi = nc.dram_tensor("fi", [128, S], F32, kind="ExternalInput")
    out = nc.dram_tensor("out", [S, C], F32, kind="ExternalOutput")

    with tile.TileContext(nc) as tc:
        with tc.tile_pool(name="persist", bufs=1) as persist:
            # persistent sbuf tensors; partition layout of qri/kri pair
            # tensors: [h(2) x duo(r,i) x 32]; one tile per 512-token chunk
            # so phase 2's dependencies resolve per chunk
            qri = [[persist.tile([128, 512], F32R, name=f"qri{p}_{c}")
                    for c in range(SC)] for p in range(2)]
            kri = [[persist.tile([128, 512], F32R, name=f"kri{p}_{c}")
                    for c in range(SC)] for p in range(2)]
            # v + ones col, split by head parity: even heads (0,2) stay
            # f32r for the exact-exp PV; odd heads (1,3) are bf16 to match
            # the bf16 Schraudolph exp bits
            vaugA = persist.tile([128, TT, 2 * 65], F32R)
            vaugD = persist.tile([128, TT, 2 * 65], BF16)
            attP = [persist.tile([128, S], F32R, name=f"attP{p}")
                    for p in range(2)]
            woP = [persist.tile([128, C], F32R, name=f"woP{p}")
                   for p in range(2)]

            # ---------------- phase 1: projections + rope ----------------
            xt_r = xt.rearrange("(kt p) s -> p kt s", p=128)
            with tc.tile_pool(name="ph1", bufs=2) as ph1, \
                 tc.tile_pool(name="ph1ps", bufs=6, space="PSUM") as ph1ps, \
                 tc.tile_pool(name="ropet", bufs=2) as ropet:
                # DMA issue order matters at startup: the first k-projection
                # needs only wkr + x chunk 0; everything else can trickle in
                # behind it.
                w_sb = {}

                def _load_w(nm, dram, width=KT * 128, pieces=1,
                            eng=None):
                    w = ph1.tile([128, width], BF16, name=f"w_{nm}",
                                 tag=nm, bufs=1)
                    step = width // pieces
                    for i in range(pieces):
                        cs = slice(step * i, step * (i + 1))
                        (eng or nc.sync).dma_start(out=w[:, cs],
                                                   in_=dram.ap()[:, cs])
                    w_sb[nm] = w

                _load_w("kr", wkr, pieces=2)
                xtqs = []
                for s0 in range(SC):
                    xtq = ph1.tile([128, KT, 512], BF16, tag="xtq", bufs=3,
                                   name=f"xtq{s0}")
                    pieces = 4 if s0 == 0 else 2
                    qeng = nc.sync
                    for i in range(pieces):
                        ks = slice(KT // pieces * i, KT // pieces * (i + 1))
                        qeng.dma_start(
                            out=xtq[:, ks, :],
                            in_=xt_r[:, ks, 512 * s0:512 * (s0 + 1)])
                    xtqs.append(xtq)
                    if s0 == 0:
                        _load_w("ki", wki)
                        _load_w("qr", wqr)
                        _load_w("qi", wqi)
                        _load_w("v", wv, width=KT * 256, pieces=2)
                    elif s0 == 1:
                        fr_sb = ph1.tile([128, S], F32, tag="fr", bufs=1)
                        fi_sb = ph1.tile([128, S], F32, tag="fi", bufs=1)
                        for i in range(2):
                            cs = slice(1024 * i, 1024 * (i + 1))
                            nc.sync.dma_start(out=fr_sb[:, cs],
                                              in_=fr.ap()[:, cs])
                            nc.sync.dma_start(out=fi_sb[:, cs],
                                              in_=fi.ap()[:, cs])
                for p in range(2):
                    nc.sync.dma_start(out=woP[p], in_=wo.ap()[p])
                # ones columns of v_aug (col 64 of each head block)
                nc.vector.memset(
                    vaugA.rearrange("p tt (h d) -> p tt h d", h=2)
                    [:, :, :, 64:65].bitcast(F32), 1.0)
                nc.vector.memset(
                    vaugD.rearrange("p tt (h d) -> p tt h d", h=2)
                    [:, :, :, 64:65], 1.0)
                # warm the exp table while phase 1 runs so phase 2 doesn't
                # pay the ACT_TABLE_LOAD + drain on its critical path
                warm = ph1.tile([1, 16], F32, tag="warm", bufs=1)
                nc.vector.memset(warm[:, 0:8], 0.0)
                nc.scalar.activation(warm[:, 8:16], warm[:, 0:8],
                                     mybir.ActivationFunctionType.Exp)

                def proj_rope(nm, dst, s0, defer_repack=False):
                    sl = slice(512 * s0, 512 * (s0 + 1))
                    xtq = xtqs[s0]
                    wr_, wi_ = w_sb[nm + "r"], w_sb[nm + "i"]
                    ps_r = ph1ps.tile([128, 512], F32, tag="proj")
                    ps_i = ph1ps.tile([128, 512], F32, tag="proj")
                    for kt in range(KT):
                        nc.tensor.matmul(
                            ps_r, wr_[:, 128 * kt:128 * (kt + 1)],
                            xtq[:, kt, :],
                            start=(kt == 0), stop=(kt == KT - 1))
                    for kt in range(KT):
                        nc.tensor.matmul(
                            ps_i, wi_[:, 128 * kt:128 * (kt + 1)],
                            xtq[:, kt, :],
                            start=(kt == 0), stop=(kt == KT - 1))
                    # rope: r' = r*fr - i*fi ; i' = r*fi + i*fr
                    t_rr = ropet.tile([128, 512], F32, tag="t0")
                    t_ii = ropet.tile([128, 512], F32, tag="t1")
                    t_ri = ropet.tile([128, 512], F32, tag="t2")
                    t_ir = ropet.tile([128, 512], F32, tag="t3")
                    nc.vector.tensor_tensor(t_rr, ps_r, fr_sb[:, sl],
                                            op=mybir.AluOpType.mult)
                    nc.vector.tensor_tensor(t_ii, ps_i, fi_sb[:, sl],
                                            op=mybir.AluOpType.mult)
                    nc.vector.tensor_tensor(t_ri, ps_r, fi_sb[:, sl],
                                            op=mybir.AluOpType.mult)
                    nc.vector.tensor_tensor(t_ir, ps_i, fr_sb[:, sl],
                                            op=mybir.AluOpType.mult)
                    c_r = ropet.tile([128, 512], F32R, tag="cr")
                    c_i = ropet.tile([128, 512], F32R, tag="ci")
                    nc.vector.tensor_tensor(c_r, t_rr, t_ii,
                                            op=mybir.AluOpType.subtract)
                    nc.vector.tensor_tensor(c_i, t_ri, t_ir,
                                            op=mybir.AluOpType.add)
                    # repack [4h x 32] r/i scratch into head-pair layout
                    def repack():
                        for p in range(2):
                            for hh in range(2):
                                h = 2 * p + hh
                                nc.sync.dma_start(
                                    out=dst[p][s0][64 * hh:64 * hh + 32, :],
                                    in_=c_r[32 * h:32 * (h + 1), :])
                                nc.sync.dma_start(
                                    out=dst[p][s0]
                                    [64 * hh + 32:64 * (hh + 1), :],
                                    in_=c_i[32 * h:32 * (h + 1), :])

                    if defer_repack:
                        return repack
                    repack()

                for s0 in range(SC):
                    if s0 == SC - 1:
                        # last chunk: q's rope first so the rope trailing
                        # into phase 2 is k3's (not needed until k-tile
                        # 12), but k3's repack DMAs queue FIRST — the
                        # in-order DMA queue must deliver k3 early while
                        # q3 isn't read until the final chunk's pass
                        q_repack = proj_rope("q", qri, s0, defer_repack=True)
                        proj_rope("k", kri, s0)
                        q_repack()
                    else:
                        proj_rope("k", kri, s0)
                        proj_rope("q", qri, s0)
                    xtq = xtqs[s0]
                    # v projection into [t, f] with ones cols interleaved
                    for tl in range(4):
                        tt = 4 * s0 + tl
                        ps_v = ph1ps.tile([128, 256], F32, tag="proj")
                        for kt in range(KT):
                            nc.tensor.matmul(
                                ps_v, xtq[:, kt, 128 * tl:128 * (tl + 1)],
                                w_sb["v"][:, 256 * kt:256 * (kt + 1)],
                                start=(kt == 0), stop=(kt == KT - 1))
                        # strided evict: head h -> cols [65h, 65h+64);
                        # ScalarE does it (idle in phase 1) so the DVE rope
                        # backlog never gates the PSUM pool handover
                        psv4 = ps_v.rearrange("p (h d) -> p h d", h=HL)
                        nc.scalar.copy(
                            vaugA[:, tt, :].rearrange(
                                "p (h d) -> p h d", h=2)[:, :, 0:64],
                            psv4[:, 0::2, :])
                        nc.scalar.copy(
                            vaugD[:, tt, :].rearrange(
                                "p (h d) -> p h d", h=2)[:, :, 0:64],
                            psv4[:, 1::2, :])

            # ------------- phase 2+3: attention + output proj -------------
            with tc.tile_pool(name="ph2", bufs=2) as ph2, \
                 tc.tile_pool(name="accps", bufs=2, space="PSUM") as accps, \
                 tc.tile_pool(name="qkps", bufs=1, space="PSUM") as qkps, \
                 tc.tile_pool(name="ph2s", bufs=3) as ph2s:

                wo_state = {}
                norm_thunks = []

                def queue_norm_quarters(s0n, ppn, accn):
                    # final-pass variant: normalize per 128-token quarter so
                    # the tail's Wo tiles start as soon as their slice lands
                    for st in range(4):
                        def tq(st=st):
                            qe = slice(128 * st, 128 * (st + 1))
                            qo = slice(512 + 128 * st, 512 + 128 * (st + 1))
                            asl = slice(512 * s0n + 128 * st,
                                        512 * s0n + 128 * (st + 1))
                            r_q = ph2s.tile([1, 256], F32, tag="recip",
                                            name=f"recq{st}")
                            nc.vector.reciprocal(r_q[:, 0:128],
                                                 accn[64:65, qe])
                            nc.vector.reciprocal(r_q[:, 128:256],
                                                 accn[64:65, qo])
                            b_q = ph2s.tile([64, 256], F32, tag="bcast",
                                            name=f"bcq{st}")
                            nc.gpsimd.partition_broadcast(b_q, r_q)
                            nc.vector.tensor_tensor(
                                attP[ppn][0:64, asl], accn[0:64, qe],
                                b_q[:, 0:128], op=mybir.AluOpType.mult)
                            att_oq = ph2s.tile([64, 128], F32R, tag="att_o",
                                               name=f"atoq{st}")
                            nc.vector.tensor_tensor(
                                att_oq, accn[0:64, qo], b_q[:, 128:256],
                                op=mybir.AluOpType.mult)
                            nc.sync.dma_start(out=attP[ppn][64:128, asl],
                                              in_=att_oq)
                        norm_thunks.append(tq)

                def queue_norm(s0n, ppn, accn):
                    # normalize attP[ppn] for chunk s0n, split into four
                    # small thunks consumed on later k-tile steps so the
                    # DVE FIFO never sees a multi-us head-of-line burst
                    ssln = slice(512 * s0n, 512 * (s0n + 1))
                    st_ = {}

                    def t1():
                        st_["r"] = ph2s.tile([1, 1024], F32, tag="recip",
                                             name=f"recip{s0n}_{ppn}")
                        nc.vector.reciprocal(st_["r"][:, 0:512],
                                             accn[64:65, 0:512])
                        st_["b"] = ph2s.tile([64, 1024], F32, tag="bcast",
                                             name=f"bcast{s0n}_{ppn}")
                        nc.gpsimd.partition_broadcast(st_["b"][:, 0:512],
                                                      st_["r"][:, 0:512])

                    def t2():
                        nc.vector.reciprocal(st_["r"][:, 512:1024],
                                             accn[64:65, 512:1024])
                        nc.gpsimd.partition_broadcast(st_["b"][:, 512:1024],
                                                      st_["r"][:, 512:1024])

                    def t3():
                        nc.vector.tensor_tensor(
                            attP[ppn][0:64, ssln], accn[0:64, 0:512],
                            st_["b"][:, 0:512], op=mybir.AluOpType.mult)

                    def t4():
                        att_o = ph2s.tile([64, 512], F32R, tag="att_o",
                                          name=f"atto{s0n}_{ppn}")
                        nc.vector.tensor_tensor(
                            att_o, accn[0:64, 512:1024],
                            st_["b"][:, 512:1024], op=mybir.AluOpType.mult)
                        nc.sync.dma_start(out=attP[ppn][64:128, ssln],
                                          in_=att_o)

                    norm_thunks.extend([t1, t2, t3, t4])

                def emit_wo_half(c0, st, p, tag="acc1", evict="act"):
                    # half of one 128-token Wo tile of chunk c0; y borrows
                    # an idle accumulator's banks (pass-1's during pass 0,
                    # pass-0's during the last chunk's pass 1), off the
                    # qk/exp critical chain. One matmul per call so each
                    # fits in the PE slack of a single k-tile step.
                    tsl = slice(512 * c0 + 128 * st, 512 * c0 + 128 * (st + 1))
                    if p == 0:
                        wo_state[st] = accps.tile([128, 1024], F32,
                                                  tag=tag, bufs=1,
                                                  name=f"y{c0}_{st}")
                    ps_y = wo_state[st]
                    for cc in range(2):
                        csl = slice(512 * cc, 512 * (cc + 1))
                        nc.tensor.matmul(ps_y[:, csl], attP[p][:, tsl],
                                         woP[p][:, csl],
                                         start=(p == 0), stop=(p == 1))
                    if p == 1:
                        y_sb = ph2.tile([128, 1024], F32, tag="y_sb", bufs=3)
                        if evict == "act":
                            nc.scalar.copy(y_sb, ps_y)
                        else:
                            nc.vector.tensor_copy(y_sb, ps_y)
                        nc.sync.dma_start(out=out.ap()[tsl, :], in_=y_sb)

                for s0 in range(SC):
                    ssl = slice(512 * s0, 512 * (s0 + 1))
                    for pp in range(2):
                        # acc halves: [0:65, 512e:512e+512] for head 2pp+e
                        acc = accps.tile([128, 1024], F32, tag=f"acc{pp}",
                                         bufs=1, name=f"acc{s0}_{pp}")
                        exps_hist = {}

                        def emit_pv(tt, exps, acc=acc, pp=pp):
                            hsl = slice(65 * pp, 65 * (pp + 1))
                            nc.tensor.matmul(
                                acc[0:65, 0:512], vaugA[:, tt, hsl],
                                exps[0],
                                start=(tt == 0), stop=(tt == TT - 1))
                            nc.tensor.matmul(
                                acc[0:65, 512:1024], vaugD[:, tt, hsl],
                                exps[1][:, :].bitcast(BF16),
                                start=(tt == 0), stop=(tt == TT - 1))

                        for tt in range(TT):
                            tc_, tof = divmod(128 * tt, 512)
                            # head-split scores in two independent 1-bank
                            # tiles: ScalarE's exact-exp chain (even head)
                            # and DVE's Schraudolph chain (odd head) never
                            # gate each other's buffer recycling
                            qkE = qkps.tile([128, 512], F32, tag="qkE",
                                            bufs=2, name=f"qkE{s0}_{pp}_{tt}")
                            qkO = qkps.tile([128, 512], F32, tag="qkO",
                                            bufs=2, name=f"qkO{s0}_{pp}_{tt}")
                            for e, qk in ((0, qkE), (1, qkO)):
                                hp = slice(64 * e, 64 * (e + 1))
                                nc.tensor.matmul(
                                    qk, kri[pp][tc_][hp, tof:tof + 128],
                                    qri[pp][s0][hp, :],
                                    start=True, stop=True,
                                    tile_position=(64 * e, 0))
                            expsA = ph2.tile([128, 512], F32R, tag="expsA",
                                             name=f"expsA{s0}_{pp}_{tt}",
                                             bufs=4)
                            expsD = ph2.tile([128, 512], mybir.dt.uint16,
                                             tag="expsD",
                                             name=f"expsD{s0}_{pp}_{tt}",
                                             bufs=4)
                            nc.scalar.activation(
                                expsA, qkE,
                                mybir.ActivationFunctionType.Exp,
                                scale=SCALE)
                            nc.vector.tensor_scalar(
                                expsD, qkO,
                                SCHRAUD_A, SCHRAUD_B,
                                op0=mybir.AluOpType.mult,
                                op1=mybir.AluOpType.add)
                            exps_hist[tt] = (expsA, expsD)
                            # PV two k-tiles behind: by the time PV enters
                            # the in-order PE queue its exp has long
                            # finished, so the queue never blocks
                            if tt > 1:
                                emit_pv(tt - 2, exps_hist.pop(tt - 2))
                            # previous pass's normalize thunks land on even
                            # k-tiles where per-tile DVE slack absorbs them
                            if norm_thunks and tt in (2, 4, 6, 8):
                                norm_thunks.pop(0)()
                            # previous chunk's Wo halves ride along pass 0,
                            # after its deferred normalize has landed
                            if pp == 0 and s0 > 0 and tt in WO_SLOTS:
                                ist = WO_SLOTS.index(tt)
                                emit_wo_half(s0 - 1, ist // 2, ist % 2)
                        emit_pv(TT - 2, exps_hist.pop(TT - 2))
                        emit_pv(TT - 1, exps_hist.pop(TT - 1))
                        if s0 == SC - 1 and pp == 1:
                            queue_norm_quarters(s0, pp, acc)
                        else:
                            queue_norm(s0, pp, acc)

                # flush the last chunk's normalize per token quarter,
                # interleaved with its Wo tiles: eight 1-bank tiles rotating
                # through the freed qk buffers (depth-2 pipelining,
                # evictions split across both engines)
                for st in range(4):
                    norm_thunks.pop(0)()
                    tsl = slice(512 * (SC - 1) + 128 * st,
                                512 * (SC - 1) + 128 * (st + 1))
                    for cc in range(2):
                        csl = slice(512 * cc, 512 * (cc + 1))
                        yt = qkps.tile([128, 512], F32,
                                       tag="qkE" if cc == 0 else "qkO",
                                       bufs=2, name=f"yt{st}_{cc}")
                        for p in range(2):
                            nc.tensor.matmul(yt, attP[p][:, tsl],
                                             woP[p][:, csl],
                                             start=(p == 0), stop=(p == 1))
                        y_sb2 = ph2.tile([128, 512], F32, tag="y_sb2",
                                         bufs=4, name=f"ysb2{st}_{cc}")
                        if (st + cc) % 2:
                            nc.scalar.copy(y_sb2, yt)
                        else:
                            nc.vector.tensor_copy(y_sb2, yt)
                        nc.sync.dma_start(out=out.ap()[tsl, csl], in_=y_sb2)

    nc.compile()
    return nc


def make_inputs(x, freqs, Wq, Wk, Wv, Wo):
    """Build the 8 per-core input maps."""
    bf = lambda a: np.ascontiguousarray(a, dtype=np.float32).astype(NP_BF16)

    # deinterleave permutations of the 256 local feature rows
    p = np.arange(128)
    real_rows = 64 * (p // 32) + 2 * (p % 32)       # within local 256 block
    imag_rows = real_rows + 1

    frh = np.ascontiguousarray(np.tile(freqs[:, :, 0].T, (HL, 1)),
                               dtype=np.float32)    # [128, S]
    fih = np.ascontiguousarray(np.tile(freqs[:, :, 1].T, (HL, 1)),
                               dtype=np.float32)

    def proj_weight(W, rows):
        # lhsT tiles: [128 c-part, KT*128], w[p, kt*128+m] = W[base+rows[m], kt*128+p]
        wt = W[rows, :]                              # [128, C]
        return bf(wt.T.reshape(KT, 128, 128).transpose(1, 0, 2)
                  .reshape(128, KT * 128))

    # one transpose+cast per batch, shared by its 4 cores
    xts = [bf(x[b].T) for b in range(B)]
    in_maps = []
    for c in range(N_CORES):
        b, hg = divmod(c, 4)
        base = 256 * hg
        wqr = proj_weight(Wq, base + real_rows)
        wqi = proj_weight(Wq, base + imag_rows)
        wkr = proj_weight(Wk, base + real_rows)
        wki = proj_weight(Wk, base + imag_rows)
        # v: [128 c-part, KT*256], wv[p, kt*256+f] = Wv[base+f, kt*128+p]
        wvt = Wv[base:base + F, :].T                 # [C, F]
        wv_ = bf(wvt.reshape(KT, 128, F).transpose(1, 0, 2)
                 .reshape(128, KT * F))
        # wo: [2, 128, C], pair p rows = Wo columns for heads 2p, 2p+1
        wo_ = np.empty((2, 128, C), np.float32)
        for pp in range(2):
            wo_[pp] = Wo[:, base + 128 * pp: base + 128 * (pp + 1)].T
        in_maps.append({
            "xt": xts[b],
            "wqr": wqr, "wqi": wqi, "wkr": wkr, "wki": wki,
            "wv": wv_, "wo": wo_.astype(np.float32),
            "fr": frh, "fi": fih,
        })
    return in_maps


def kernel(x, freqs, Wq, Wk, Wv, Wo):
    global _CACHED_NC
    x = np.asarray(x, dtype=np.float32)
    freqs = np.asarray(freqs, dtype=np.float32)
    Wq = np.asarray(Wq, dtype=np.float32)
    Wk = np.asarray(Wk, dtype=np.float32)
    Wv = np.asarray(Wv, dtype=np.float32)
    Wo = np.asarray(Wo, dtype=np.float32)

    in_maps = make_inputs(x, freqs, Wq, Wk, Wv, Wo)
    if _CACHED_NC is None:
        _CACHED_NC = build_module()
    res = run_bass_kernel_spmd(_CACHED_NC, in_maps,
                               core_ids=list(range(N_CORES)))
    outs = [r["out"] for r in res.results]
    y = np.empty((B, S, C), np.float32)
    for b in range(B):
        y[b] = outs[4 * b] + outs[4 * b + 1] + outs[4 * b + 2] + outs[4 * b + 3]
    return y


if __name__ == "__main__":
    rng = np.random.default_rng(0)
    x = rng.standard_normal((B, S, C)).astype(np.float32)
    freqs = rng.standard_normal((S, DH, 2)).astype(np.float32)
    ws = [(rng.standard_normal((C, C)) * C ** -0.5).astype(np.float32)
          for _ in range(4)]
    y = kernel(x, freqs, *ws)
    print("out", y.shape, y.dtype, float(np.abs(y).mean()))
